# revision 12
# baseline (speedup 1.0000x reference)
"""Chamfer distance kernel for 8 Trainium2 NeuronCores (Bass/Tile).

Problem: xyz1, xyz2: (4, 8192, 3) fp32. Outputs dist1, dist2: (4, 8192) fp32,
the row-wise / column-wise minima of the pairwise squared-distance matrix
d[n,m] = max(||x_n||^2 + ||y_m||^2 - 2 x_n.y_m, 0), per batch.

Sharding: core c handles batch c//2 and half of the N rows (c%2). Each core
computes dist1 for its 4096 rows exactly, and a dist2 partial (min over its
4096 rows) for all 8192 columns; the host min-combines the two partials.

Per-core kernel: distance tiles are produced by ONE bf16 matmul each, using
K=24 augmented vectors: bf16x3 decompositions of x, of -2*y and of the two
squared norms, ordered so the large terms cancel early in the fp32 PSUM
accumulation (fp32-faithful, representation residual ~2^-27).

Both outputs are row-min reductions of some orientation of d, so the kernel
computes BOTH orientations (d = xa^T ya and d^T = ya^T xa — the TensorEngine
has large headroom) in [128, 4x512] PSUM groups (4 banks), and the DVE does a
single tensor_reduce(min) per group. dist1 comes from the A-orientation
reduces, dist2 from the B-orientation ones. Group partials land in columns of
a small SBUF tile; one strided reduce + relu finishes each output.
"""

from contextlib import ExitStack

import numpy as np
import ml_dtypes

B, N, M = 4, 8192, 8192
NCORES = 8
NLOC = N // 2          # rows of xyz1 per core
P = 128                # partitions
FD = 512               # matmul free dim (one PSUM bank of fp32)
GRP = 4                # m-chunks per PSUM group (4 banks = [128, 2048])
KAUG = 24

_BF16 = ml_dtypes.bfloat16


def _decomp3(v):
    """fp32/fp64 array -> three bf16 planes summing to v (residual ~2^-27)."""
    v = v.astype(np.float32)
    h = v.astype(_BF16)
    r = v - h.astype(np.float32)
    m = r.astype(_BF16)
    r2 = r - m.astype(np.float32)
    l = r2.astype(_BF16)
    return h, m, l


def _build_aug(x, y):
    """x: [Nl,3] fp32, y: [Mm,3] fp32 -> (xa [KAUG,Nl] bf16, ya [KAUG,Mm] bf16).

    d[n,m] = sum_k xa[k,n]*ya[k,m] up to bf16x3 residuals. Slot order puts the
    large mutually-cancelling terms first so fp32 PSUM accumulation stays
    accurate near d ~ 0.
    """
    nl, mm = x.shape[0], y.shape[0]
    nx = (x.astype(np.float64) ** 2).sum(axis=1)
    ny = (y.astype(np.float64) ** 2).sum(axis=1)
    xh, xm, xl = _decomp3(x)
    y2 = (-2.0 * y.astype(np.float64)).astype(np.float32)
    yh, ym, yl = _decomp3(y2)
    nxh, nxm, nxl = _decomp3(nx)
    nyh, nym, nyl = _decomp3(ny)

    one_n = np.ones(nl, dtype=_BF16)
    one_m = np.ones(mm, dtype=_BF16)

    xa = np.empty((KAUG, nl), dtype=_BF16)
    ya = np.empty((KAUG, mm), dtype=_BF16)
    k = 0

    def slot(xv, yv):
        nonlocal k
        xa[k] = xv
        ya[k] = yv
        k += 1

    slot(nxh, one_m)
    slot(one_n, nyh)
    for c in range(3):
        slot(xh[:, c], yh[:, c])
    slot(nxm, one_m)
    slot(one_n, nym)
    for c in range(3):
        slot(xh[:, c], ym[:, c])
    for c in range(3):
        slot(xm[:, c], yh[:, c])
    slot(nxl, one_m)
    slot(one_n, nyl)
    for c in range(3):
        slot(xh[:, c], yl[:, c])
    for c in range(3):
        slot(xm[:, c], ym[:, c])
    for c in range(3):
        slot(xl[:, c], yh[:, c])
    assert k == KAUG
    return xa, ya


def build_bass(nloc=NLOC, m_total=M, repeat=1, grp=None, psum_bufs=2):
    """Build + compile the per-core Bass program.

    repeat>1 wraps the main compute in a dynamic loop executing it `repeat`
    times — used only to measure per-iteration HW time above the PJRT
    dispatch noise floor.
    """
    import concourse.bacc as bacc
    import concourse.tile as tile
    import concourse.mybir as mybir

    f32 = mybir.dt.float32
    bf16 = mybir.dt.bfloat16
    Alu = mybir.AluOpType
    X = mybir.AxisListType.X
    XY = mybir.AxisListType.XY

    grp = GRP if grp is None else grp
    ntile_a = nloc // P          # weight tiles, orientation A (dist1 rows)
    ngrp_a = m_total // (grp * FD)   # reduce groups per A weight tile
    ntile_b = m_total // P       # weight tiles, orientation B (dist2 rows)
    ngrp_b = nloc // (grp * FD)      # reduce groups per B weight tile

    nc = bacc.Bacc("TRN2", target_bir_lowering=False, debug=False)
    xa_d = nc.dram_tensor("xa", [KAUG, nloc], bf16, kind="ExternalInput")
    ya_d = nc.dram_tensor("ya", [KAUG, m_total], bf16, kind="ExternalInput")
    d1_d = nc.dram_tensor("d1", [P, ntile_a], f32, kind="ExternalOutput")
    d2_d = nc.dram_tensor("d2", [P, ntile_b], f32, kind="ExternalOutput")

    with tile.TileContext(nc) as tc, ExitStack() as ctx:
        singles = ctx.enter_context(tc.tile_pool(name="singles", bufs=1))
        psum = ctx.enter_context(
            tc.tile_pool(name="psum", bufs=psum_bufs, space="PSUM")
        )

        xa = singles.tile([KAUG, nloc], bf16)
        nc.sync.dma_start(out=xa, in_=xa_d.ap())
        ya = singles.tile([KAUG, m_total], bf16)
        nc.sync.dma_start(out=ya, in_=ya_d.ap())

        c1 = singles.tile([P, ntile_a * ngrp_a], f32)
        c2 = singles.tile([P, ntile_b * ngrp_b], f32)
        d1t = singles.tile([P, ntile_a], f32)
        d2t = singles.tile([P, ntile_b], f32)

        def orientation(wt_count, grp_count, w_sb, mv_sb, cols):
            """One orientation: wt_count weight tiles x grp_count reduce
            groups; rows of the output come from w_sb, the reduction runs
            over all of mv_sb."""
            for it in range(wt_count):
                for g in range(grp_count):
                    pt = psum.tile([P, grp, FD], f32, name="pt", tag="pt")
                    for j in range(grp):
                        nc.tensor.matmul(
                            pt[:, j, :],
                            w_sb[:, it * P : (it + 1) * P],
                            mv_sb[:, (g * grp + j) * FD : (g * grp + j + 1) * FD],
                            start=True,
                            stop=True,
                        )
                    col = it * grp_count + g
                    nc.vector.tensor_reduce(
                        out=cols[:, col : col + 1], in_=pt, axis=XY, op=Alu.min
                    )

        def main_compute():
            orientation(ntile_a, ngrp_a, xa, ya, c1)
            orientation(ntile_b, ngrp_b, ya, xa, c2)

        if repeat == 1:
            main_compute()
        else:
            with tc.For_i(0, repeat, 1):
                main_compute()

        # finals: strided min over each weight-tile's group partials + relu
        nc.vector.tensor_reduce(
            out=d1t,
            in_=c1.rearrange("p (t g) -> p t g", g=ngrp_a),
            axis=X,
            op=Alu.min,
        )
        nc.vector.tensor_reduce(
            out=d2t,
            in_=c2.rearrange("p (t g) -> p t g", g=ngrp_b),
            axis=X,
            op=Alu.min,
        )
        nc.vector.tensor_scalar_max(out=d1t, in0=d1t, scalar1=0.0)
        nc.vector.tensor_scalar_max(out=d2t, in0=d2t, scalar1=0.0)

        nc.sync.dma_start(out=d1_d.ap(), in_=d1t)
        nc.sync.dma_start(out=d2_d.ap(), in_=d2t)

    nc.compile()
    return nc


_CACHED_NC = None


def _get_nc():
    global _CACHED_NC
    if _CACHED_NC is None:
        _CACHED_NC = build_bass()
    return _CACHED_NC


def _make_in_maps(xyz1, xyz2):
    xyz1 = np.asarray(xyz1, dtype=np.float32)
    xyz2 = np.asarray(xyz2, dtype=np.float32)
    in_maps = []
    for c in range(NCORES):
        b, h = divmod(c, 2)
        x = xyz1[b, h * NLOC : (h + 1) * NLOC]
        y = xyz2[b]
        xa, ya = _build_aug(x, y)
        in_maps.append({"xa": xa, "ya": ya})
    return in_maps


def _unshard(results):
    dist1 = np.empty((B, N), np.float32)
    dist2 = np.empty((B, M), np.float32)
    for c in range(NCORES):
        b, h = divmod(c, 2)
        dist1[b, h * NLOC : (h + 1) * NLOC] = np.asarray(results[c]["d1"]).T.ravel()
        d2p = np.asarray(results[c]["d2"]).T.ravel()
        if h == 0:
            dist2[b] = d2p
        else:
            np.minimum(dist2[b], d2p, out=dist2[b])
    return dist1, dist2


def kernel(xyz1, xyz2):
    from concourse.bass_utils import run_bass_kernel_spmd

    nc = _get_nc()
    in_maps = _make_in_maps(xyz1, xyz2)
    res = run_bass_kernel_spmd(nc, in_maps, core_ids=list(range(NCORES)))
    return _unshard(res.results)
